# revision 11
# baseline (speedup 1.0000x reference)
"""Multi-head dilated sliding-window attention (window=129, dil=1) on 8 TRN2 cores.

Sharding: sequence-parallel. Each core computes 256 query rows (N=2048 / 8),
with a 64-row K/V halo on each side (zero-padded at the sequence edges).
Weights are replicated (resident in SBUF, bf16).

Band-softmax identity (reference softmaxes the FULL row with zeros outside
the band):
    out_i = (sum_band (e^{s_ij} - 1) V_j + sum_all V_j) / (sum_band (e^{s_ij} - 1) + N)
with V_raw = x@Wv (bv folded into bo' = bo + bv@Wo on the host), bk applied
only to real (non-padding) K rows via an indicator-row matmul, and the global
row  [sum_all V_j | N]  precomputed on the host (biascat).

v4 structure:
  - x arrives pre-transposed (xT layout); Wq/Wk arrive db-major so round db
    depends on one 256KB tile
  - DMA split across rings by issue cost: scalar ring carries the early tiles
    (xT, Wv, db0 pair, identity, small biases) before ACT's compute starts;
    sync ring carries db-pairs 1-7 + Wo; gpsimd the round-1 consts
  - per-head-pair rounds: Q/K proj -> scores (head pair on PE row groups
    0-63/64-127, interleaved -> concurrent) -> exp (ACT) -> fused -1/mask
    (DVE) -> PV of previous round -> zinv scales (DVE) -> per-round PE
    transposes of the finished A block (epilogue balanced: ACT=exp/copies,
    DVE=everything else)
  - output projection at the end; contraction blocks 0-5 overlap the last
    flush; out DMA'd as bf16
"""

import numpy as np
import ml_dtypes
from contextlib import ExitStack

import concourse.bass as bass
import concourse.tile as tile
from concourse import bacc, mybir
from concourse.bass_utils import run_bass_kernel_spmd

F32 = mybir.dt.float32
BF16 = mybir.dt.bfloat16
NPBF16 = ml_dtypes.bfloat16
N, E, H, D = 2048, 1024, 16, 64
R = N // 8          # 256 query rows per core
HALO = R + 128      # 384 K/V rows per core
NQB = R // 128      # query blocks per core


def build_graph():
    nc = bacc.Bacc("TRN2", target_bir_lowering=False, debug=False, num_devices=8)

    xt_d = nc.declare_dram_parameter("xhT", [E, HALO], BF16, isOutput=False)
    xvalid_d = nc.declare_dram_parameter("xvalid", [1, HALO], BF16, isOutput=False)
    wq_d = nc.declare_dram_parameter("Wq_db", [8, 128, H * D], BF16,
                                     isOutput=False)
    wk_d = nc.declare_dram_parameter("Wk_db", [8, 128, H * D], BF16,
                                     isOutput=False)
    wv_d = nc.declare_dram_parameter("Wv", [E, H * D], BF16, isOutput=False)
    wo_d = nc.declare_dram_parameter("Wo", [H * D, E], BF16, isOutput=False)
    bq_d = nc.declare_dram_parameter("bq_r", [128, 8], F32, isOutput=False)
    bk_d = nc.declare_dram_parameter("bk_row", [1, H * D], BF16, isOutput=False)
    bo_d = nc.declare_dram_parameter("bo_row", [1, E], BF16, isOutput=False)
    bc_d = nc.declare_dram_parameter("biascat_r", [1, H * (D + 1)], BF16,
                                     isOutput=False)
    m4_d = nc.declare_dram_parameter("mask4", [128, 512], BF16, isOutput=False)
    id_d = nc.declare_dram_parameter("ident", [128, 128], BF16, isOutput=False)
    out_d = nc.declare_dram_parameter("out", [R, E], BF16, isOutput=True)

    with tile.TileContext(nc) as tc, ExitStack() as ctx:
        const = ctx.enter_context(tc.tile_pool(name="const", bufs=1))
        pers = ctx.enter_context(tc.tile_pool(name="pers", bufs=1))
        epool = ctx.enter_context(tc.tile_pool(name="epool", bufs=3))
        ppool = ctx.enter_context(tc.tile_pool(name="ppool", bufs=5))
        zpool = ctx.enter_context(tc.tile_pool(name="zpool", bufs=4))
        obpool = ctx.enter_context(tc.tile_pool(name="obpool", bufs=2))
        psum = ctx.enter_context(tc.tile_pool(name="psum", bufs=8, space="PSUM"))

        def ps(shape, dt=F32):
            return psum.tile(shape, dt, tag="ps", name="pst")

        # ---- PE warm-up emitted first: dummy matmuls keep the PE busy while
        # the first DMAs land, so HAM is at 8/8 when real work starts.
        wu = const.tile([128, 512], BF16, tag="wu")
        nc.vector.memset(wu[:], 0.0)
        wups = psum.tile([128, 512], F32, tag="ps", name="wups")
        for _ in range(14):
            nc.tensor.matmul(wups[:], wu[:, 0:128], wu[:], start=True, stop=True)

        # ---- loads.  scalar ring: everything needed before ~round 1 (ACT is
        # idle until then); sync ring: db-pairs 1-7 + Wo; gpsimd: consts
        # needed from round 1 (Q7 issue is slow but off the critical engines).
        xT = pers.tile([128, 8, HALO], BF16, tag="xT")       # [e_p, e_t, seq]
        wv_t = const.tile([128, 8, E], BF16, tag="wv")
        wo_t = const.tile([128, 8, E], BF16, tag="wo")
        wq_t = [const.tile([128, H * D], BF16, tag=f"wq{db}", name="wt")
                for db in range(8)]
        wk_t = [const.tile([128, H * D], BF16, tag=f"wk{db}", name="wt")
                for db in range(8)]
        m4 = const.tile([128, 512], BF16, tag="m4")
        bq_sb = const.tile([128, 8], F32, tag="bq")
        bk_sb = const.tile([1, H * D], BF16, tag="bk")
        bo_sb = const.tile([1, E], BF16, tag="bo")
        bc_sb = const.tile([1, H, D + 1], BF16, tag="bc")
        valid_sb = const.tile([1, HALO], BF16, tag="valid")
        identity = const.tile([128, 128], BF16, tag="identity")

        nc.scalar.dma_start(xT[:],
                            xt_d.rearrange("(t p) s -> p t s", p=128))
        nc.scalar.dma_start(wv_t[:],
                            wv_d.rearrange("(t p) d -> p t d", p=128))
        nc.scalar.dma_start(wq_t[0][:], wq_d[0])
        nc.scalar.dma_start(wk_t[0][:], wk_d[0])
        nc.scalar.dma_start(bq_sb[:], bq_d[:, :])
        nc.scalar.dma_start(bk_sb[:], bk_d[:, :])
        nc.scalar.dma_start(valid_sb[:], xvalid_d[:, :])
        nc.scalar.dma_start(identity[:], id_d[:, :])
        for db in range(1, 8):
            nc.sync.dma_start(wq_t[db][:], wq_d[db])
            nc.sync.dma_start(wk_t[db][:], wk_d[db])
        nc.sync.dma_start(wo_t[:],
                          wo_d.rearrange("(t p) d -> p t d", p=128))
        nc.sync.dma_start(bo_sb[:], bo_d[:, :])
        nc.gpsimd.dma_start(m4[:], m4_d[:, :])
        nc.gpsimd.dma_start(bc_sb[:].rearrange("o h d -> o (h d)"), bc_d[:, :])
        ones_sb = const.tile([1, 128], BF16, tag="ones")
        nc.vector.memset(ones_sb[:], 1.0)

        # ---- persistent activations ---------------------------------------
        QT = pers.tile([128, 8, R], BF16, tag="QT")          # [d_p, d_t, q]
        KT = pers.tile([128, 8, HALO], BF16, tag="KT")       # [d_p, d_t, seq]
        Vaug = pers.tile([128, 3, H, D + 1], BF16, tag="Vaug")
        Asc = pers.tile([128, NQB, H * D], BF16, tag="Asc")  # [q_p, qblk, dims]
        AT = pers.tile([128, 8, R], BF16, tag="AT")          # [d_p, d_t, q]

        # ---- V projection, et-outer so it paces with the Wv DMA stream ----
        vps = [ps([128, 512]) for _ in range(6)]             # [st*2+hf]
        for et in range(8):
            for st in range(3):
                for hf in range(2):
                    nc.tensor.matmul(vps[st * 2 + hf][:],
                                     xT[:, et, st * 128:(st + 1) * 128],
                                     wv_t[:, et, hf * 512:(hf + 1) * 512],
                                     start=(et == 0), stop=(et == 7))
        for st in range(3):
            for hf in range(2):
                src = vps[st * 2 + hf][:].rearrange("p (h d) -> p h d", d=D)
                nc.scalar.copy(Vaug[:, st, hf * 8:(hf + 1) * 8, 0:D], src)
        nc.vector.memset(Vaug[:, :, :, D:D + 1], 1.0)

        # ---- fused projections + banded attention, one head-pair at a time
        # round r = db (one head pair, BOTH query blocks). Emission order:
        #   proj(r) -> PV+scales of r-1 -> S(r) -> PE transposes of r-1
        # Per-head p layout: [q0c0 | q0c1 | q1c0 | q1c1], quadrant j uses
        # keys halo block (qblk+cblk) and mask m0/m1 alternating.
        prev = None  # (db, ptiles{h: pt})

        wqv = [wq_t[db][:].rearrange("p (t d) -> p t d", d=128) for db in range(8)]
        wkv = [wk_t[db][:].rearrange("p (t d) -> p t d", d=128) for db in range(8)]

        def proj(db):
            qp = ps([128, R])
            for et in range(8):
                nc.tensor.matmul(qp[:], wqv[db][:, et, :],
                                 xT[:, et, 64:64 + R],
                                 start=(et == 0), stop=(et == 7))
            nc.scalar.add(QT[:, db, :], qp[:], bq_sb[:, db:db + 1])
            kp = ps([128, HALO])
            for et in range(8):
                nc.tensor.matmul(kp[:], wkv[db][:, et, :],
                                 xT[:, et, :], start=(et == 0), stop=False)
            nc.tensor.matmul(kp[:], bk_sb[0:1, db * 128:(db + 1) * 128],
                             valid_sb[0:1, :], start=False, stop=True)
            nc.scalar.copy(KT[:, db, :], kp[:])

        def pv_flush(pr):
            """PV matmuls + normalization scales for head pair db (one bank)."""
            db, ptl = pr
            pv = ps([128, NQB, 2 * (D + 1)])
            for qblk in range(NQB):
                for i, h in enumerate((2 * db, 2 * db + 1)):
                    off = i * (D + 1)
                    for cblk in range(2):
                        quad = qblk * 2 + cblk
                        nc.tensor.matmul(pv[:, qblk, off:off + D + 1],
                                         ptl[h][:, quad * 128:(quad + 1) * 128],
                                         Vaug[:, qblk + cblk, h, :],
                                         start=(qblk == 0 and i == 0 and cblk == 0),
                                         stop=False)
            bc_pair = bc_sb[0:1, 2 * db:2 * db + 2, :]
            for qblk in range(NQB):
                nc.tensor.matmul(pv[:, qblk, :], ones_sb[0:1, :],
                                 bc_pair.rearrange("o h d -> o (h d)"),
                                 start=False, stop=(qblk == NQB - 1))
            for qblk in range(NQB):
                for i, h in enumerate((2 * db, 2 * db + 1)):
                    off = i * (D + 1)
                    zinv = zpool.tile([128, 1], F32, tag="z", name="zinv")
                    nc.vector.reciprocal(zinv[:], pv[:, qblk, off + D:off + D + 1])
                    nc.vector.tensor_scalar_mul(
                        Asc[:, qblk, h * D:(h + 1) * D],
                        pv[:, qblk, off:off + D], zinv[:])

        def a_transpose(db):
            """Asc block of head pair db -> AT via PE transpose."""
            for qblk in range(NQB):
                tp = ps([128, 128], BF16)
                nc.tensor.transpose(tp[:], Asc[:, qblk, db * 128:(db + 1) * 128],
                                    identity[:])
                nc.vector.tensor_copy(AT[:, db, qblk * 128:(qblk + 1) * 128],
                                      tp[:])

        def outproj(at_list, start, stop):
            for at in at_list:
                for qblk in range(NQB):
                    for hf in range(2):
                        nc.tensor.matmul(ops[qblk * 2 + hf][:],
                                         AT[:, at, qblk * 128:(qblk + 1) * 128],
                                         wo_t[:, at, hf * 512:(hf + 1) * 512],
                                         start=(start and at == at_list[0]),
                                         stop=False)
            if stop:
                for qblk in range(NQB):
                    for hf in range(2):
                        nc.tensor.matmul(ops[qblk * 2 + hf][:], ones_sb[0:1, :],
                                         bo_sb[0:1, hf * 512:(hf + 1) * 512],
                                         start=False, stop=True)

        for r in range(8 + 1):
            if r < 8:
                db = r
                proj(db)
                if prev is not None:
                    pv_flush(prev)
                # S matmuls: head A on PE rows 0-63, head B on rows 64-127 —
                # interleaved emission so the two row-groups run concurrently.
                sps = {h: ps([128, 512]) for h in (2 * db, 2 * db + 1)}
                for quad in range(4):
                    qblk, cblk = quad // 2, quad % 2
                    for i, h in enumerate((2 * db, 2 * db + 1)):
                        rr = i * 64
                        nc.tensor.matmul(
                            sps[h][:, quad * 128:(quad + 1) * 128],
                            KT[rr:rr + 64, db,
                               (qblk + cblk) * 128:(qblk + cblk + 1) * 128],
                            QT[rr:rr + 64, db, qblk * 128:(qblk + 1) * 128],
                            start=(quad == 0), stop=(quad == 3))
                if prev is not None:
                    a_transpose(prev[0])
                ptl = {}
                for h in (2 * db, 2 * db + 1):
                    et_ = epool.tile([128, 512], F32, tag="e", name="et_")
                    nc.scalar.activation(et_[:], sps[h][:],
                                         mybir.ActivationFunctionType.Exp)
                    pt = ppool.tile([128, 512], BF16, tag="p", name="pt")
                    nc.vector.scalar_tensor_tensor(
                        pt[:], et_[:], -1.0, m4[:],
                        mybir.AluOpType.add, mybir.AluOpType.mult)
                    ptl[h] = pt
                prev = (db, ptl)
            else:
                # tail: contraction blocks 0-5 of the output projection run
                # while the last head pair's PV/scale chain completes.
                ops = [ps([128, 512]) for _ in range(2 * NQB)]
                outproj([0, 1, 2, 3, 4, 5], start=True, stop=False)
                pv_flush(prev)
                a_transpose(prev[0])
                outproj([6, 7], start=False, stop=True)

        for qblk in range(NQB):
            ob = obpool.tile([128, E], BF16, tag="ob")
            for hf in range(2):
                nc.vector.tensor_copy(ob[:, hf * 512:(hf + 1) * 512],
                                      ops[qblk * 2 + hf][:])
            nc.scalar.dma_start(out_d[qblk * 128:(qblk + 1) * 128, :], ob[:])

    nc.compile()
    return nc


_NC = None


def get_nc():
    global _NC
    if _NC is None:
        _NC = build_graph()
    return _NC


def make_in_maps(x, Wq, bq, Wk, bk, Wv, bv, Wo, bo):
    f = lambda a: np.ascontiguousarray(np.asarray(a, dtype=np.float32))
    bf = lambda a: np.ascontiguousarray(
        np.asarray(a, dtype=np.float32).astype(NPBF16))
    x2 = f(x).reshape(N, E)
    Wv32, Wo32 = f(Wv), f(Wo)
    ci = np.arange(128, dtype=np.float32)[:, None]  # key index c (partitions)
    qi = np.arange(128, dtype=np.float32)[None, :]  # query index q (free)
    m0 = (ci >= qi).astype(np.float32)
    m1 = (ci <= qi).astype(np.float32)
    mask4 = np.concatenate([m0, m1, m0, m1], axis=1)
    # host-folded epilogue bias: bo' = bo + bv @ Wo
    bo_row = (f(bo) + f(bv) @ Wo32).reshape(1, E)
    # host-computed global-sum row: per head [sum_n V_n | N]
    sv = (x2.sum(0, dtype=np.float32) @ Wv32).reshape(H, D)
    biascat = np.concatenate(
        [sv, np.full((H, 1), float(N), np.float32)], axis=1).reshape(1, -1)

    # db-major Wq/Wk: dbm[db, e_part, et*128+d] = W[et*128+e_part, db*128+d]
    def dbm(W):
        return np.ascontiguousarray(
            f(W).reshape(8, 128, 8, 128).transpose(2, 1, 0, 3)
            .reshape(8, 128, H * D).astype(NPBF16))
    common = {
        "Wq_db": dbm(Wq), "Wk_db": dbm(Wk), "Wv": bf(Wv), "Wo": bf(Wo),
        "bq_r": f(bq).reshape(8, 128).T.copy(),
        "bk_row": bf(bk).reshape(1, H * D),
        "bo_row": bf(bo_row),
        "biascat_r": bf(biascat),
        "mask4": bf(mask4),
        "ident": np.eye(128, dtype=np.float32).astype(NPBF16),
    }
    in_maps = []
    for c in range(8):
        r0 = c * R
        xh = np.zeros((HALO, E), np.float32)
        valid = np.zeros((1, HALO), NPBF16)
        lo, hi = r0 - 64, r0 + R + 64
        slo, shi = max(lo, 0), min(hi, N)
        xh[slo - lo: shi - lo] = x2[slo:shi]
        valid[0, slo - lo: shi - lo] = 1.0
        xhT = np.ascontiguousarray(xh.T.astype(NPBF16))
        in_maps.append({**common, "xhT": xhT, "xvalid": valid})
    return in_maps


def kernel(x, Wq, bq, Wk, bk, Wv, bv, Wo, bo, _trace=False, _trace_kwargs=None):
    nc = get_nc()
    in_maps = make_in_maps(x, Wq, bq, Wk, bk, Wv, bv, Wo, bo)
    res = run_bass_kernel_spmd(nc, in_maps, list(range(8)), trace=_trace,
                               **(_trace_kwargs or {}))
    out = np.concatenate([res.results[c]["out"] for c in range(8)], axis=0)
    kernel.last_result = res
    return out[None].astype(np.float32)


# revision 16
# speedup vs baseline: 1.4194x; 1.4194x over previous
"""Multi-head dilated sliding-window attention (window=129, dil=1) on 8 TRN2 cores.

Sharding: sequence-parallel. Each core computes 256 query rows (N=2048 / 8),
with a 64-row K/V halo on each side (zero-padded at the sequence edges).
Weights are replicated (resident in SBUF, bf16).

Band-softmax identity (reference softmaxes the FULL row with zeros outside
the band):
    out_i = (sum_band (e^{s_ij} - 1) V_j + sum_all V_j) / (sum_band (e^{s_ij} - 1) + N)
with V_raw = x@Wv (bv folded into bo' = bo + bv@Wo on the host), bk applied
only to real (non-padding) K rows via an indicator-row matmul, and the global
row  [sum_all V_j | N]  precomputed on the host (biascat).

v4 structure:
  - x arrives pre-transposed (xT layout); Wq/Wk arrive db-major so round db
    depends on one 256KB tile
  - DMA split across rings by issue cost: scalar ring carries the early tiles
    (xT, Wv, db0 pair, identity, small biases) before ACT's compute starts;
    sync ring carries db-pairs 1-7 + Wo; gpsimd the round-1 consts
  - per-head-pair rounds: Q/K proj -> scores (head pair on PE row groups
    0-63/64-127, interleaved -> concurrent) -> exp (ACT) -> fused -1/mask
    (DVE) -> PV of previous round -> zinv scales (DVE) -> per-round PE
    transposes of the finished A block (epilogue balanced: ACT=exp/copies,
    DVE=everything else)
  - output projection at the end; contraction blocks 0-5 overlap the last
    flush; out DMA'd as bf16
"""

import numpy as np
import ml_dtypes
from contextlib import ExitStack

import concourse.bass as bass
import concourse.tile as tile
from concourse import bacc, mybir
from concourse.bass_utils import run_bass_kernel_spmd

F32 = mybir.dt.float32
BF16 = mybir.dt.bfloat16
NPBF16 = ml_dtypes.bfloat16
N, E, H, D = 2048, 1024, 16, 64
R = N // 8          # 256 query rows per core
HALO = R + 128      # 384 K/V rows per core
NQB = R // 128      # query blocks per core


def build_graph():
    nc = bacc.Bacc("TRN2", target_bir_lowering=False, debug=False, num_devices=8)

    xt_d = nc.declare_dram_parameter("xhT", [E, HALO], BF16, isOutput=False)
    xvalid_d = nc.declare_dram_parameter("xvalid", [1, HALO], BF16, isOutput=False)
    wq_d = nc.declare_dram_parameter("Wq_db", [8, 128, H * D], BF16,
                                     isOutput=False)
    wk_d = nc.declare_dram_parameter("Wk_db", [8, 128, H * D], BF16,
                                     isOutput=False)
    wv_d = nc.declare_dram_parameter("Wv", [E, H * D], BF16, isOutput=False)
    wo_d = nc.declare_dram_parameter("Wo", [H * D, E], BF16, isOutput=False)
    bq_d = nc.declare_dram_parameter("bq_r", [128, 8], F32, isOutput=False)
    bk_d = nc.declare_dram_parameter("bk_row", [1, H * D], BF16, isOutput=False)
    bo_d = nc.declare_dram_parameter("bo_row", [1, E], BF16, isOutput=False)
    bc_d = nc.declare_dram_parameter("biascat_r", [1, H * (D + 1)], BF16,
                                     isOutput=False)
    m4_d = nc.declare_dram_parameter("mask4", [128, 512], BF16, isOutput=False)
    id_d = nc.declare_dram_parameter("ident", [128, 128], BF16, isOutput=False)
    out_d = nc.declare_dram_parameter("out", [R, E], BF16, isOutput=True)

    with tile.TileContext(nc) as tc, ExitStack() as ctx:
        const = ctx.enter_context(tc.tile_pool(name="const", bufs=1))
        pers = ctx.enter_context(tc.tile_pool(name="pers", bufs=1))
        epool = ctx.enter_context(tc.tile_pool(name="epool", bufs=3))
        ppool = ctx.enter_context(tc.tile_pool(name="ppool", bufs=5))
        zpool = ctx.enter_context(tc.tile_pool(name="zpool", bufs=4))
        obpool = ctx.enter_context(tc.tile_pool(name="obpool", bufs=2))
        psum = ctx.enter_context(tc.tile_pool(name="psum", bufs=8, space="PSUM"))

        def ps(shape, dt=F32):
            return psum.tile(shape, dt, tag="ps", name="pst")

        # ---- PE warm-up emitted first: dummy matmuls keep the PE busy while
        # the first DMAs land, so HAM is at 8/8 when real work starts.
        wu = const.tile([128, 512], BF16, tag="wu")
        nc.vector.memset(wu[:], 0.0)
        wups = psum.tile([128, 512], F32, tag="ps", name="wups")
        for _ in range(14):
            nc.tensor.matmul(wups[:], wu[:, 0:128], wu[:], start=True, stop=True)

        # ---- loads: one sync-ring FIFO in need order (xT, Wv, db pairs with
        # small consts early, Wo last).  Per-tile transfers — big batched
        # rearranged APs measured slower (descriptor patterns).
        xT = pers.tile([128, 8, HALO], BF16, tag="xT")       # [e_p, e_t, seq]
        for et in range(8):
            nc.sync.dma_start(xT[:, et, :], xt_d[et * 128:(et + 1) * 128, :])

        wv_t = [const.tile([128, E], BF16, tag=f"wv{et}", name="wt")
                for et in range(8)]
        wo_t = [const.tile([128, E], BF16, tag=f"wo{et}", name="wt")
                for et in range(8)]
        wq_t = [const.tile([128, H * D], BF16, tag=f"wq{db}", name="wt")
                for db in range(8)]
        wk_t = [const.tile([128, H * D], BF16, tag=f"wk{db}", name="wt")
                for db in range(8)]
        m4 = const.tile([128, 512], BF16, tag="m4")
        bq_sb = const.tile([128, 8], F32, tag="bq")
        bk_sb = const.tile([1, H * D], BF16, tag="bk")
        bo_sb = const.tile([1, E], BF16, tag="bo")
        bc_sb = const.tile([1, H, D + 1], BF16, tag="bc")
        valid_sb = const.tile([1, HALO], BF16, tag="valid")
        identity = const.tile([128, 128], BF16, tag="identity")

        for et in range(8):
            nc.sync.dma_start(wv_t[et][:], wv_d[et * 128:(et + 1) * 128, :])
        nc.sync.dma_start(wq_t[0][:], wq_d[0])
        nc.sync.dma_start(wk_t[0][:], wk_d[0])
        nc.sync.dma_start(bq_sb[:], bq_d[:, :])
        nc.sync.dma_start(bk_sb[:], bk_d[:, :])
        nc.sync.dma_start(valid_sb[:], xvalid_d[:, :])
        nc.sync.dma_start(identity[:], id_d[:, :])
        nc.sync.dma_start(m4[:], m4_d[:, :])
        nc.sync.dma_start(bc_sb[:].rearrange("o h d -> o (h d)"), bc_d[:, :])
        for db in range(1, 8):
            nc.sync.dma_start(wq_t[db][:], wq_d[db])
            nc.sync.dma_start(wk_t[db][:], wk_d[db])
        for et in range(8):
            nc.sync.dma_start(wo_t[et][:], wo_d[et * 128:(et + 1) * 128, :])
        nc.sync.dma_start(bo_sb[:], bo_d[:, :])
        ones_sb = const.tile([1, 128], BF16, tag="ones")
        nc.vector.memset(ones_sb[:], 1.0)

        # ---- persistent activations ---------------------------------------
        QT = pers.tile([128, 8, R], BF16, tag="QT")          # [d_p, d_t, q]
        KT = pers.tile([128, 8, HALO], BF16, tag="KT")       # [d_p, d_t, seq]
        Vaug = pers.tile([128, 3, H, D + 1], BF16, tag="Vaug")
        Asc = pers.tile([128, NQB, H * D], BF16, tag="Asc")  # [q_p, qblk, dims]
        AT = pers.tile([128, 8, R], BF16, tag="AT")          # [d_p, d_t, q]

        # ---- V projection, et-outer so it paces with the Wv DMA stream ----
        vps = [ps([128, 512]) for _ in range(6)]             # [st*2+hf]
        for et in range(8):
            for st in range(3):
                for hf in range(2):
                    nc.tensor.matmul(vps[st * 2 + hf][:],
                                     xT[:, et, st * 128:(st + 1) * 128],
                                     wv_t[et][:, hf * 512:(hf + 1) * 512],
                                     start=(et == 0), stop=(et == 7))
        for st in range(3):
            for hf in range(2):
                src = vps[st * 2 + hf][:].rearrange("p (h d) -> p h d", d=D)
                nc.scalar.copy(Vaug[:, st, hf * 8:(hf + 1) * 8, 0:D], src)
        nc.vector.memset(Vaug[:, :, :, D:D + 1], 1.0)

        # ---- fused projections + banded attention, one head-pair at a time
        # round r = db (one head pair, BOTH query blocks). Emission order:
        #   proj(r) -> PV+scales of r-1 -> S(r) -> PE transposes of r-1
        # Per-head p layout: [q0c0 | q0c1 | q1c0 | q1c1], quadrant j uses
        # keys halo block (qblk+cblk) and mask m0/m1 alternating.
        prev = None  # (db, ptiles{h: pt})

        wqv = [wq_t[db][:].rearrange("p (t d) -> p t d", d=128) for db in range(8)]
        wkv = [wk_t[db][:].rearrange("p (t d) -> p t d", d=128) for db in range(8)]

        def proj(db):
            qp = ps([128, R])
            for et in range(8):
                nc.tensor.matmul(qp[:], wqv[db][:, et, :],
                                 xT[:, et, 64:64 + R],
                                 start=(et == 0), stop=(et == 7))
            nc.scalar.add(QT[:, db, :], qp[:], bq_sb[:, db:db + 1])
            kp = ps([128, HALO])
            for et in range(8):
                nc.tensor.matmul(kp[:], wkv[db][:, et, :],
                                 xT[:, et, :], start=(et == 0), stop=False)
            nc.tensor.matmul(kp[:], bk_sb[0:1, db * 128:(db + 1) * 128],
                             valid_sb[0:1, :], start=False, stop=True)
            nc.scalar.copy(KT[:, db, :], kp[:])

        def pv_flush(pr):
            """PV matmuls + normalization scales for head pair db (one bank)."""
            db, ptl = pr
            pv = ps([128, NQB, 2 * (D + 1)])
            for qblk in range(NQB):
                for i, h in enumerate((2 * db, 2 * db + 1)):
                    off = i * (D + 1)
                    for cblk in range(2):
                        quad = qblk * 2 + cblk
                        nc.tensor.matmul(pv[:, qblk, off:off + D + 1],
                                         ptl[h][:, quad * 128:(quad + 1) * 128],
                                         Vaug[:, qblk + cblk, h, :],
                                         start=(qblk == 0 and i == 0 and cblk == 0),
                                         stop=False)
            bc_pair = bc_sb[0:1, 2 * db:2 * db + 2, :]
            for qblk in range(NQB):
                nc.tensor.matmul(pv[:, qblk, :], ones_sb[0:1, :],
                                 bc_pair.rearrange("o h d -> o (h d)"),
                                 start=False, stop=(qblk == NQB - 1))
            for qblk in range(NQB):
                for i, h in enumerate((2 * db, 2 * db + 1)):
                    off = i * (D + 1)
                    zinv = zpool.tile([128, 1], F32, tag="z", name="zinv")
                    nc.vector.reciprocal(zinv[:], pv[:, qblk, off + D:off + D + 1])
                    nc.vector.tensor_scalar_mul(
                        Asc[:, qblk, h * D:(h + 1) * D],
                        pv[:, qblk, off:off + D], zinv[:])

        def a_transpose(db):
            """Asc block of head pair db -> AT via PE transpose."""
            for qblk in range(NQB):
                tp = ps([128, 128], BF16)
                nc.tensor.transpose(tp[:], Asc[:, qblk, db * 128:(db + 1) * 128],
                                    identity[:])
                nc.scalar.copy(AT[:, db, qblk * 128:(qblk + 1) * 128], tp[:])

        def outproj(at_list, start, stop):
            for at in at_list:
                for qblk in range(NQB):
                    for hf in range(2):
                        nc.tensor.matmul(ops[qblk * 2 + hf][:],
                                         AT[:, at, qblk * 128:(qblk + 1) * 128],
                                         wo_t[at][:, hf * 512:(hf + 1) * 512],
                                         start=(start and at == at_list[0]),
                                         stop=False)
            if stop:
                for qblk in range(NQB):
                    for hf in range(2):
                        nc.tensor.matmul(ops[qblk * 2 + hf][:], ones_sb[0:1, :],
                                         bo_sb[0:1, hf * 512:(hf + 1) * 512],
                                         start=False, stop=True)

        for r in range(8 + 1):
            if r < 8:
                db = r
                proj(db)
                if prev is not None:
                    pv_flush(prev)
                # S matmuls: head A on PE rows 0-63, head B on rows 64-127 —
                # interleaved emission so the two row-groups run concurrently.
                sps = {h: ps([128, 512]) for h in (2 * db, 2 * db + 1)}
                for quad in range(4):
                    qblk, cblk = quad // 2, quad % 2
                    for i, h in enumerate((2 * db, 2 * db + 1)):
                        rr = i * 64
                        nc.tensor.matmul(
                            sps[h][:, quad * 128:(quad + 1) * 128],
                            KT[rr:rr + 64, db,
                               (qblk + cblk) * 128:(qblk + cblk + 1) * 128],
                            QT[rr:rr + 64, db, qblk * 128:(qblk + 1) * 128],
                            start=(quad == 0), stop=(quad == 3))
                # transpose trails the flush by a full round so the PE never
                # waits on the DVE normalization chain
                if r >= 2:
                    a_transpose(r - 2)
                ptl = {}
                for h in (2 * db, 2 * db + 1):
                    et_ = epool.tile([128, 512], F32, tag="e", name="et_")
                    nc.scalar.activation(et_[:], sps[h][:],
                                         mybir.ActivationFunctionType.Exp)
                    pt = ppool.tile([128, 512], BF16, tag="p", name="pt")
                    nc.vector.scalar_tensor_tensor(
                        pt[:], et_[:], -1.0, m4[:],
                        mybir.AluOpType.add, mybir.AluOpType.mult)
                    ptl[h] = pt
                prev = (db, ptl)
            else:
                # tail: contraction blocks 0-5 of the output projection run
                # while the last head pair's PV/scale chain completes.
                ops = [ps([128, 512]) for _ in range(2 * NQB)]
                outproj([0, 1, 2, 3, 4, 5], start=True, stop=False)
                pv_flush(prev)
                a_transpose(6)
                a_transpose(7)
                outproj([6, 7], start=False, stop=True)

        for qblk in range(NQB):
            ob = obpool.tile([128, E], BF16, tag="ob")
            for hf in range(2):
                nc.vector.tensor_copy(ob[:, hf * 512:(hf + 1) * 512],
                                      ops[qblk * 2 + hf][:])
            nc.scalar.dma_start(out_d[qblk * 128:(qblk + 1) * 128, :], ob[:])

    nc.compile()
    return nc


_NC = None


def get_nc():
    global _NC
    if _NC is None:
        _NC = build_graph()
    return _NC


def make_in_maps(x, Wq, bq, Wk, bk, Wv, bv, Wo, bo):
    f = lambda a: np.ascontiguousarray(np.asarray(a, dtype=np.float32))
    bf = lambda a: np.ascontiguousarray(
        np.asarray(a, dtype=np.float32).astype(NPBF16))
    x2 = f(x).reshape(N, E)
    Wv32, Wo32 = f(Wv), f(Wo)
    ci = np.arange(128, dtype=np.float32)[:, None]  # key index c (partitions)
    qi = np.arange(128, dtype=np.float32)[None, :]  # query index q (free)
    m0 = (ci >= qi).astype(np.float32)
    m1 = (ci <= qi).astype(np.float32)
    mask4 = np.concatenate([m0, m1, m0, m1], axis=1)
    # host-folded epilogue bias: bo' = bo + bv @ Wo
    bo_row = (f(bo) + f(bv) @ Wo32).reshape(1, E)
    # host-computed global-sum row: per head [sum_n V_n | N]
    sv = (x2.sum(0, dtype=np.float32) @ Wv32).reshape(H, D)
    biascat = np.concatenate(
        [sv, np.full((H, 1), float(N), np.float32)], axis=1).reshape(1, -1)

    # db-major Wq/Wk: dbm[db, e_part, et*128+d] = W[et*128+e_part, db*128+d]
    def dbm(W):
        return np.ascontiguousarray(
            f(W).reshape(8, 128, 8, 128).transpose(2, 1, 0, 3)
            .reshape(8, 128, H * D).astype(NPBF16))
    common = {
        "Wq_db": dbm(Wq), "Wk_db": dbm(Wk), "Wv": bf(Wv), "Wo": bf(Wo),
        "bq_r": f(bq).reshape(8, 128).T.copy(),
        "bk_row": bf(bk).reshape(1, H * D),
        "bo_row": bf(bo_row),
        "biascat_r": bf(biascat),
        "mask4": bf(mask4),
        "ident": np.eye(128, dtype=np.float32).astype(NPBF16),
    }
    in_maps = []
    for c in range(8):
        r0 = c * R
        xh = np.zeros((HALO, E), np.float32)
        valid = np.zeros((1, HALO), NPBF16)
        lo, hi = r0 - 64, r0 + R + 64
        slo, shi = max(lo, 0), min(hi, N)
        xh[slo - lo: shi - lo] = x2[slo:shi]
        valid[0, slo - lo: shi - lo] = 1.0
        xhT = np.ascontiguousarray(xh.T.astype(NPBF16))
        in_maps.append({**common, "xhT": xhT, "xvalid": valid})
    return in_maps


def kernel(x, Wq, bq, Wk, bk, Wv, bv, Wo, bo, _trace=False, _trace_kwargs=None):
    nc = get_nc()
    in_maps = make_in_maps(x, Wq, bq, Wk, bk, Wv, bv, Wo, bo)
    res = run_bass_kernel_spmd(nc, in_maps, list(range(8)), trace=_trace,
                               **(_trace_kwargs or {}))
    out = np.concatenate([res.results[c]["out"] for c in range(8)], axis=0)
    kernel.last_result = res
    return out[None].astype(np.float32)


# revision 17
# speedup vs baseline: 1.5171x; 1.0688x over previous
"""Multi-head dilated sliding-window attention (window=129, dil=1) on 8 TRN2 cores.

Sharding: sequence-parallel. Each core computes 256 query rows (N=2048 / 8),
with a 64-row K/V halo on each side (zero-padded at the sequence edges).
Weights are replicated (resident in SBUF, bf16).

Band-softmax identity (reference softmaxes the FULL row with zeros outside
the band):
    out_i = (sum_band (e^{s_ij} - 1) V_j + sum_all V_j) / (sum_band (e^{s_ij} - 1) + N)
with V_raw = x@Wv (bv folded into bo' = bo + bv@Wo on the host), bk applied
only to real (non-padding) K rows via an indicator-row matmul, and the global
row  [sum_all V_j | N]  precomputed on the host (biascat).

v6 structure:
  - all inputs arrive in device layout from the host, packed into a few large
    contiguous blobs (DMA issue on the sync engine costs ~0.6us per
    dma_start, so few big transfers beat many small ones; 2D-contiguous APs
    keep descriptors at full rate)
  - per-head-pair rounds with a 2-deep software pipeline:
      round r: Q/K proj(r) | PV+normalize(r-2) | scores(r) | A-transpose(r-3)
    scores run the head pair concurrently on PE row groups 0-63/64-127;
    exp on ACT; fused (e-1)*mask, one reciprocal over all four z columns and
    two broadcast-scale ops on DVE; A transposes on the PE trail far enough
    that they never wait on the DVE chain
  - output projection at the end; contraction blocks 0-5 overlap the last
    two flushes; out DMA'd as bf16
"""

import numpy as np
import ml_dtypes
from contextlib import ExitStack

import concourse.bass as bass
import concourse.tile as tile
from concourse import bacc, mybir
from concourse.bass_utils import run_bass_kernel_spmd

F32 = mybir.dt.float32
BF16 = mybir.dt.bfloat16
NPBF16 = ml_dtypes.bfloat16
N, E, H, D = 2048, 1024, 16, 64
R = N // 8          # 256 query rows per core
HALO = R + 128      # 384 K/V rows per core
NQB = R // 128      # query blocks per core
RB = E + HALO + H * (D + 1) + E   # packed row-consts: bk|valid|biascat|bo


def build_graph():
    nc = bacc.Bacc("TRN2", target_bir_lowering=False, debug=False, num_devices=8)

    xt_d = nc.declare_dram_parameter("xhT", [128, 8 * HALO], BF16, isOutput=False)
    wv_d = nc.declare_dram_parameter("Wv_r", [128, 8 * E], BF16, isOutput=False)
    wo_d = nc.declare_dram_parameter("Wo_r", [128, 8 * E], BF16, isOutput=False)
    # r0 blob: wq0 | wk0 | m4 | ident | bq  (bq in bf16)
    r0_d = nc.declare_dram_parameter("r0blob", [128, 2 * H * D + 512 + 128 + 8],
                                     BF16, isOutput=False)
    qk_d = nc.declare_dram_parameter("QKdb", [7, 128, 2 * H * D], BF16,
                                     isOutput=False)
    row_d = nc.declare_dram_parameter("rowc", [1, RB], BF16, isOutput=False)
    out_d = nc.declare_dram_parameter("out", [R, E], BF16, isOutput=True)

    with tile.TileContext(nc) as tc, ExitStack() as ctx:
        const = ctx.enter_context(tc.tile_pool(name="const", bufs=1))
        pers = ctx.enter_context(tc.tile_pool(name="pers", bufs=1))
        epool = ctx.enter_context(tc.tile_pool(name="epool", bufs=3))
        ppool = ctx.enter_context(tc.tile_pool(name="ppool", bufs=6))
        zpool = ctx.enter_context(tc.tile_pool(name="zpool", bufs=4))
        obpool = ctx.enter_context(tc.tile_pool(name="obpool", bufs=2))
        psum = ctx.enter_context(tc.tile_pool(name="psum", bufs=8, space="PSUM"))

        def ps(shape, dt=F32):
            return psum.tile(shape, dt, tag="ps", name="pst")

        # ---- PE warm-up: dummy matmuls from t~0 so HAM reaches 8/8 before
        # real work; sized to end as the first weight blob lands.
        wu = const.tile([128, 512], BF16, tag="wu")
        nc.vector.memset(wu[:], 0.0)
        wups = psum.tile([128, 512], F32, tag="ps", name="wups")
        for _ in range(11):
            nc.tensor.matmul(wups[:], wu[:, 0:128], wu[:], start=True, stop=True)

        # ---- loads: one sync-ring FIFO of big contiguous blobs ------------
        xT = pers.tile([128, 8, HALO], BF16, tag="xT")       # [e_p, e_t, seq]
        wv_t = const.tile([128, 8, E], BF16, tag="wv")       # [e_p, e_t, d]
        wo_t = const.tile([128, 8, E], BF16, tag="wo")       # [d_p, d_t, e]
        r0_t = const.tile([128, 2 * H * D + 512 + 128 + 8], BF16, tag="r0")
        qk_t = [const.tile([128, 2 * H * D], BF16, tag=f"qk{db}", name="qk")
                for db in range(8)]
        row_t = const.tile([1, RB], BF16, tag="rowc")

        xv = xT[:].rearrange("p t s -> p (t s)")
        nc.sync.dma_start(xv, xt_d[:, :])
        wvv = wv_t[:].rearrange("p t d -> p (t d)")
        nc.sync.dma_start(wvv[:, 0:4 * E], wv_d[:, 0:4 * E])
        nc.sync.dma_start(wvv[:, 4 * E:8 * E], wv_d[:, 4 * E:8 * E])
        nc.sync.dma_start(r0_t[:], r0_d[:, :])
        nc.sync.dma_start(row_t[:], row_d[:, :])
        for db in range(1, 8):
            nc.sync.dma_start(qk_t[db][:], qk_d[db - 1])
        wov = wo_t[:].rearrange("p t d -> p (t d)")
        nc.sync.dma_start(wov, wo_d[:, :])

        # views into the blobs
        qk_views = []
        for db in range(8):
            base = r0_t[:] if db == 0 else qk_t[db][:]
            qkv = base[:, 0:2 * H * D].rearrange("p (w t d) -> p w t d",
                                                 w=2, d=128)
            qk_views.append(qkv)
        m4 = r0_t[:, 2 * H * D:2 * H * D + 512]
        identity = r0_t[:, 2 * H * D + 512:2 * H * D + 640]
        bq_sb = r0_t[:, 2 * H * D + 640:2 * H * D + 648]
        bk_sb = row_t[0:1, 0:E]
        valid_sb = row_t[0:1, E:E + HALO]
        bc_sb = row_t[0:1, E + HALO:E + HALO + H * (D + 1)].rearrange(
            "o (h d) -> o h d", d=D + 1)
        bo_sb = row_t[0:1, E + HALO + H * (D + 1):RB]
        ones_sb = const.tile([1, 128], BF16, tag="ones")
        nc.vector.memset(ones_sb[:], 1.0)

        # ---- persistent activations ---------------------------------------
        QT = pers.tile([128, 8, R], BF16, tag="QT")          # [d_p, d_t, q]
        KT = pers.tile([128, 8, HALO], BF16, tag="KT")       # [d_p, d_t, seq]
        Vaug = pers.tile([128, 3, H, D + 1], BF16, tag="Vaug")
        Asc = pers.tile([128, NQB, H * D], BF16, tag="Asc")  # [q_p, qblk, dims]
        AT = pers.tile([128, 8, R], BF16, tag="AT")          # [d_p, d_t, q]

        # ---- V projection, et-outer so it paces with the Wv DMA stream ----
        vps = [ps([128, 512]) for _ in range(6)]             # [st*2+hf]
        for et in range(8):
            for st in range(3):
                for hf in range(2):
                    nc.tensor.matmul(vps[st * 2 + hf][:],
                                     xT[:, et, st * 128:(st + 1) * 128],
                                     wv_t[:, et, hf * 512:(hf + 1) * 512],
                                     start=(et == 0), stop=(et == 7))
        for st in range(3):
            for hf in range(2):
                src = vps[st * 2 + hf][:].rearrange("p (h d) -> p h d", d=D)
                nc.scalar.copy(Vaug[:, st, hf * 8:(hf + 1) * 8, 0:D], src)
        nc.vector.memset(Vaug[:, :, :, D:D + 1], 1.0)

        def proj(db):
            qp = ps([128, R])
            for et in range(8):
                nc.tensor.matmul(qp[:], qk_views[db][:, 0, et, :],
                                 xT[:, et, 64:64 + R],
                                 start=(et == 0), stop=(et == 7))
            nc.scalar.add(QT[:, db, :], qp[:], bq_sb[:, db:db + 1])
            kp = ps([128, HALO])
            for et in range(8):
                nc.tensor.matmul(kp[:], qk_views[db][:, 1, et, :],
                                 xT[:, et, :], start=(et == 0), stop=False)
            nc.tensor.matmul(kp[:], bk_sb[0:1, db * 128:(db + 1) * 128],
                             valid_sb[0:1, :], start=False, stop=True)
            nc.scalar.copy(KT[:, db, :], kp[:])

        def pv_flush(pr):
            """PV matmuls + normalization scales for head pair db (one bank)."""
            db, ptl = pr
            pv = ps([128, NQB, 2 * (D + 1)])
            for qblk in range(NQB):
                for i, h in enumerate((2 * db, 2 * db + 1)):
                    off = i * (D + 1)
                    for cblk in range(2):
                        quad = qblk * 2 + cblk
                        nc.tensor.matmul(pv[:, qblk, off:off + D + 1],
                                         ptl[h][:, quad * 128:(quad + 1) * 128],
                                         Vaug[:, qblk + cblk, h, :],
                                         start=(qblk == 0 and i == 0 and cblk == 0),
                                         stop=False)
            bc_pair = bc_sb[0:1, 2 * db:2 * db + 2, :]
            for qblk in range(NQB):
                nc.tensor.matmul(pv[:, qblk, :], ones_sb[0:1, :],
                                 bc_pair.rearrange("o h d -> o (h d)"),
                                 start=False, stop=(qblk == NQB - 1))
            # one reciprocal over all four z columns, two broadcast scales
            zinv = zpool.tile([128, NQB, 2], F32, tag="z", name="zinv")
            nc.vector.reciprocal(zinv[:], pv[:, :, D::D + 1])
            for qblk in range(NQB):
                src = pv[:, qblk, :].rearrange("p (w d) -> p w d", d=D + 1)
                nc.vector.tensor_tensor(
                    Asc[:, qblk, 2 * db * D:(2 * db + 2) * D]
                    .rearrange("p (w d) -> p w d", d=D),
                    src[:, :, 0:D],
                    zinv[:, qblk, :, None].to_broadcast([128, 2, D]),
                    mybir.AluOpType.mult)

        def a_transpose(db):
            """Asc block of head pair db -> AT via PE transpose."""
            for qblk in range(NQB):
                tp = ps([128, 128], BF16)
                nc.tensor.transpose(tp[:], Asc[:, qblk, db * 128:(db + 1) * 128],
                                    identity[:])
                nc.scalar.copy(AT[:, db, qblk * 128:(qblk + 1) * 128], tp[:])

        def outproj(at_list, start, stop):
            for at in at_list:
                for qblk in range(NQB):
                    for hf in range(2):
                        nc.tensor.matmul(ops[qblk * 2 + hf][:],
                                         AT[:, at, qblk * 128:(qblk + 1) * 128],
                                         wo_t[:, at, hf * 512:(hf + 1) * 512],
                                         start=(start and at == at_list[0]),
                                         stop=False)
            if stop:
                for qblk in range(NQB):
                    for hf in range(2):
                        nc.tensor.matmul(ops[qblk * 2 + hf][:], ones_sb[0:1, :],
                                         bo_sb[0:1, hf * 512:(hf + 1) * 512],
                                         start=False, stop=True)

        def scores(db):
            # head A on PE rows 0-63, head B on rows 64-127 — interleaved
            # emission so the two row-groups run concurrently.
            sps = {h: ps([128, 512]) for h in (2 * db, 2 * db + 1)}
            for quad in range(4):
                qblk, cblk = quad // 2, quad % 2
                for i, h in enumerate((2 * db, 2 * db + 1)):
                    rr = i * 64
                    nc.tensor.matmul(
                        sps[h][:, quad * 128:(quad + 1) * 128],
                        KT[rr:rr + 64, db,
                           (qblk + cblk) * 128:(qblk + cblk + 1) * 128],
                        QT[rr:rr + 64, db, qblk * 128:(qblk + 1) * 128],
                        start=(quad == 0), stop=(quad == 3))
            ptl = {}
            for h in (2 * db, 2 * db + 1):
                et_ = epool.tile([128, 512], F32, tag="e", name="et_")
                nc.scalar.activation(et_[:], sps[h][:],
                                     mybir.ActivationFunctionType.Exp)
                pt = ppool.tile([128, 512], BF16, tag="p", name="pt")
                nc.vector.scalar_tensor_tensor(
                    pt[:], et_[:], -1.0, m4[:],
                    mybir.AluOpType.add, mybir.AluOpType.mult)
                ptl[h] = pt
            return ptl

        pend = {}   # db -> ptl, awaiting flush
        for r in range(8):
            proj(r)
            if r >= 2:
                pv_flush((r - 2, pend.pop(r - 2)))
            pend[r] = scores(r)
            if r >= 3:
                a_transpose(r - 3)

        # tail: flush 6 and 7; output projection blocks 0-5 overlap them
        ops = [ps([128, 512]) for _ in range(2 * NQB)]
        pv_flush((6, pend.pop(6)))
        a_transpose(5)
        outproj([0, 1, 2, 3, 4, 5], start=True, stop=False)
        a_transpose(6)
        pv_flush((7, pend.pop(7)))
        outproj([6], start=False, stop=False)
        a_transpose(7)
        outproj([7], start=False, stop=True)

        for qblk in range(NQB):
            ob = obpool.tile([128, E], BF16, tag="ob")
            for hf in range(2):
                nc.vector.tensor_copy(ob[:, hf * 512:(hf + 1) * 512],
                                      ops[qblk * 2 + hf][:])
            nc.scalar.dma_start(out_d[qblk * 128:(qblk + 1) * 128, :], ob[:])

    nc.compile()
    return nc


_NC = None


def get_nc():
    global _NC
    if _NC is None:
        _NC = build_graph()
    return _NC


def make_in_maps(x, Wq, bq, Wk, bk, Wv, bv, Wo, bo):
    f = lambda a: np.ascontiguousarray(np.asarray(a, dtype=np.float32))
    bf = lambda a: np.ascontiguousarray(
        np.asarray(a, dtype=np.float32).astype(NPBF16))
    x2 = f(x).reshape(N, E)
    Wv32, Wo32 = f(Wv), f(Wo)
    ci = np.arange(128, dtype=np.float32)[:, None]  # key index c (partitions)
    qi = np.arange(128, dtype=np.float32)[None, :]  # query index q (free)
    m0 = (ci >= qi).astype(np.float32)
    m1 = (ci <= qi).astype(np.float32)
    mask4 = np.concatenate([m0, m1, m0, m1], axis=1)
    # host-folded epilogue bias: bo' = bo + bv @ Wo
    bo_row = (f(bo) + f(bv) @ Wo32).reshape(1, E)
    # host-computed global-sum row: per head [sum_n V_n | N]
    sv = (x2.sum(0, dtype=np.float32) @ Wv32).reshape(H, D)
    biascat = np.concatenate(
        [sv, np.full((H, 1), float(N), np.float32)], axis=1).reshape(1, -1)

    # db-major W: dbm[db, e_part, et*128+d] = W[et*128+e_part, db*128+d]
    def dbm(W):
        return (f(W).reshape(8, 128, 8, 128).transpose(2, 1, 0, 3)
                .reshape(8, 128, H * D))
    wqm, wkm = dbm(Wq), dbm(Wk)
    qkdb = np.concatenate([wqm, wkm], axis=2)       # [db, 128, 2048]
    r0blob = np.concatenate(
        [qkdb[0], mask4, np.eye(128, dtype=np.float32),
         f(bq).reshape(8, 128).T], axis=1)
    # [e_p, et*HALO] / [p, et*E] device layouts
    def etmaj(W):                                   # [E, X] -> [128, 8*X]
        Wf = f(W)
        return (Wf.reshape(8, 128, Wf.shape[1]).transpose(1, 0, 2)
                .reshape(128, -1))
    rowc = np.concatenate(
        [f(bk).reshape(1, E), np.zeros((1, HALO), np.float32),
         biascat, bo_row], axis=1)
    common = {
        "Wv_r": bf(etmaj(Wv)), "Wo_r": bf(etmaj(Wo)),
        "r0blob": bf(r0blob), "QKdb": bf(qkdb[1:]),
    }
    in_maps = []
    for c in range(8):
        r0 = c * R
        xh = np.zeros((HALO, E), np.float32)
        valid = np.zeros((1, HALO), np.float32)
        lo, hi = r0 - 64, r0 + R + 64
        slo, shi = max(lo, 0), min(hi, N)
        xh[slo - lo: shi - lo] = x2[slo:shi]
        valid[0, slo - lo: shi - lo] = 1.0
        rc = rowc.copy()
        rc[0, E:E + HALO] = valid
        xhT = etmaj(xh.T)                           # [128, 8*HALO]
        in_maps.append({**common, "xhT": bf(xhT), "rowc": bf(rc)})
    return in_maps


def kernel(x, Wq, bq, Wk, bk, Wv, bv, Wo, bo, _trace=False, _trace_kwargs=None):
    nc = get_nc()
    in_maps = make_in_maps(x, Wq, bq, Wk, bk, Wv, bv, Wo, bo)
    res = run_bass_kernel_spmd(nc, in_maps, list(range(8)), trace=_trace,
                               **(_trace_kwargs or {}))
    out = np.concatenate([res.results[c]["out"] for c in range(8)], axis=0)
    kernel.last_result = res
    return out[None].astype(np.float32)


# revision 20
# speedup vs baseline: 1.5635x; 1.0305x over previous
"""Multi-head dilated sliding-window attention (window=129, dil=1) on 8 TRN2 cores.

Sharding: sequence-parallel. Each core computes 256 query rows (N=2048 / 8),
with a 64-row K/V halo on each side (zero-padded at the sequence edges).
Weights are replicated (resident in SBUF, bf16).

Band-softmax identity (reference softmaxes the FULL row with zeros outside
the band):
    out_i = (sum_band (e^{s_ij} - 1) V_j + sum_all V_j) / (sum_band (e^{s_ij} - 1) + N)
with V_raw = x@Wv (bv folded into bo' = bo + bv@Wo on the host), bk applied
only to real (non-padding) K rows via an indicator-row matmul, and the global
row  [sum_all V_j | N]  precomputed on the host (biascat).

v6 structure:
  - all inputs arrive in device layout from the host, packed into a few large
    contiguous blobs (DMA issue on the sync engine costs ~0.6us per
    dma_start, so few big transfers beat many small ones; 2D-contiguous APs
    keep descriptors at full rate)
  - per-head-pair rounds with a 2-deep software pipeline:
      round r: Q/K proj(r) | PV+normalize(r-2) | scores(r) | A-transpose(r-3)
    scores run the head pair concurrently on PE row groups 0-63/64-127;
    exp on ACT; fused (e-1)*mask, one reciprocal over all four z columns and
    two broadcast-scale ops on DVE; A transposes on the PE trail far enough
    that they never wait on the DVE chain
  - output projection at the end; contraction blocks 0-5 overlap the last
    two flushes; out DMA'd as bf16
"""

import numpy as np
import ml_dtypes
from contextlib import ExitStack

import concourse.bass as bass
import concourse.tile as tile
from concourse import bacc, mybir
from concourse.bass_utils import run_bass_kernel_spmd

F32 = mybir.dt.float32
BF16 = mybir.dt.bfloat16
NPBF16 = ml_dtypes.bfloat16
N, E, H, D = 2048, 1024, 16, 64
R = N // 8          # 256 query rows per core
HALO = R + 128      # 384 K/V rows per core
NQB = R // 128      # query blocks per core
RB = E + HALO + H * (D + 1) + E   # packed row-consts: bk|valid|biascat|bo


def build_graph():
    nc = bacc.Bacc("TRN2", target_bir_lowering=False, debug=False, num_devices=8)

    xt_d = nc.declare_dram_parameter("xhT", [128, 8 * HALO], BF16, isOutput=False)
    wv_d = nc.declare_dram_parameter("Wv_r", [128, 8 * E], BF16, isOutput=False)
    wo_d = nc.declare_dram_parameter("Wo_r", [128, 8 * E], BF16, isOutput=False)
    # r0 blob: wq0 | wk0 | m4 | ident | bq  (bq in bf16)
    r0_d = nc.declare_dram_parameter("r0blob", [128, 2 * H * D + 512 + 128 + 8],
                                     BF16, isOutput=False)
    qk_d = nc.declare_dram_parameter("QKdb", [7, 128, 2 * H * D], BF16,
                                     isOutput=False)
    row_d = nc.declare_dram_parameter("rowc", [1, RB], BF16, isOutput=False)
    out_d = nc.declare_dram_parameter("out", [R, E], BF16, isOutput=True)

    with tile.TileContext(nc) as tc, ExitStack() as ctx:
        const = ctx.enter_context(tc.tile_pool(name="const", bufs=1))
        pers = ctx.enter_context(tc.tile_pool(name="pers", bufs=1))
        epool = ctx.enter_context(tc.tile_pool(name="epool", bufs=3))
        ppool = ctx.enter_context(tc.tile_pool(name="ppool", bufs=6))
        zpool = ctx.enter_context(tc.tile_pool(name="zpool", bufs=4))
        obpool = ctx.enter_context(tc.tile_pool(name="obpool", bufs=2))
        psum = ctx.enter_context(tc.tile_pool(name="psum", bufs=8, space="PSUM"))

        def ps(shape, dt=F32):
            return psum.tile(shape, dt, tag="ps", name="pst")

        # ---- PE warm-up: dummy matmuls from t~0 so HAM reaches 8/8 before
        # real work; sized to end as the first weight blob lands.
        wu = const.tile([128, 512], BF16, tag="wu")
        nc.vector.memset(wu[:], 0.0)
        wups = psum.tile([128, 512], F32, tag="ps", name="wups")
        for _ in range(11):
            nc.tensor.matmul(wups[:], wu[:, 0:128], wu[:], start=True, stop=True)

        # ---- loads: one sync-ring FIFO of big contiguous blobs ------------
        xT = pers.tile([128, 8, HALO], BF16, tag="xT")       # [e_p, e_t, seq]
        wv_t = const.tile([128, 8, E], BF16, tag="wv")       # [e_p, e_t, d]
        wo_t = const.tile([128, 8, E], BF16, tag="wo")       # [d_p, d_t, e]
        r0_t = const.tile([128, 2 * H * D + 512 + 128 + 8], BF16, tag="r0")
        qk_t = [const.tile([128, 2 * H * D], BF16, tag=f"qk{db}", name="qk")
                for db in range(8)]
        row_t = const.tile([1, RB], BF16, tag="rowc")

        xv = xT[:].rearrange("p t s -> p (t s)")
        nc.sync.dma_start(xv[:, 0:4 * HALO], xt_d[:, 0:4 * HALO])
        wvv = wv_t[:].rearrange("p t d -> p (t d)")
        nc.sync.dma_start(wvv[:, 0:2 * E], wv_d[:, 0:2 * E])
        nc.sync.dma_start(xv[:, 4 * HALO:8 * HALO], xt_d[:, 4 * HALO:8 * HALO])
        nc.sync.dma_start(wvv[:, 2 * E:4 * E], wv_d[:, 2 * E:4 * E])
        nc.sync.dma_start(wvv[:, 4 * E:6 * E], wv_d[:, 4 * E:6 * E])
        nc.sync.dma_start(wvv[:, 6 * E:8 * E], wv_d[:, 6 * E:8 * E])
        nc.sync.dma_start(r0_t[:], r0_d[:, :])
        nc.sync.dma_start(row_t[:], row_d[:, :])
        for db in range(1, 8):
            nc.sync.dma_start(qk_t[db][:], qk_d[db - 1])
        wov = wo_t[:].rearrange("p t d -> p (t d)")
        nc.sync.dma_start(wov, wo_d[:, :])

        # views into the blobs
        qk_views = []
        for db in range(8):
            base = r0_t[:] if db == 0 else qk_t[db][:]
            qkv = base[:, 0:2 * H * D].rearrange("p (w t d) -> p w t d",
                                                 w=2, d=128)
            qk_views.append(qkv)
        m4 = r0_t[:, 2 * H * D:2 * H * D + 512]
        identity = r0_t[:, 2 * H * D + 512:2 * H * D + 640]
        bq_sb = r0_t[:, 2 * H * D + 640:2 * H * D + 648]
        bk_sb = row_t[0:1, 0:E]
        valid_sb = row_t[0:1, E:E + HALO]
        bc_sb = row_t[0:1, E + HALO:E + HALO + H * (D + 1)].rearrange(
            "o (h d) -> o h d", d=D + 1)
        bo_sb = row_t[0:1, E + HALO + H * (D + 1):RB]
        ones_sb = const.tile([1, 128], BF16, tag="ones")
        nc.vector.memset(ones_sb[:], 1.0)

        # ---- persistent activations ---------------------------------------
        QT = pers.tile([128, 8, R], BF16, tag="QT")          # [d_p, d_t, q]
        KT = pers.tile([128, 8, HALO], BF16, tag="KT")       # [d_p, d_t, seq]
        Vaug = pers.tile([128, 3, H, D + 1], BF16, tag="Vaug")
        Asc = pers.tile([128, NQB, H * D], BF16, tag="Asc")  # [q_p, qblk, dims]
        AT = pers.tile([128, 8, R], BF16, tag="AT")          # [d_p, d_t, q]

        # ---- V projection, et-outer so it paces with the Wv DMA stream ----
        vps = [ps([128, 512]) for _ in range(6)]             # [st*2+hf]
        for et in range(8):
            for st in range(3):
                for hf in range(2):
                    nc.tensor.matmul(vps[st * 2 + hf][:],
                                     xT[:, et, st * 128:(st + 1) * 128],
                                     wv_t[:, et, hf * 512:(hf + 1) * 512],
                                     start=(et == 0), stop=(et == 7))
        # Vaug copies on DVE: ACT must stay free for round 0/1's QT/KT copies
        for st in range(3):
            for hf in range(2):
                src = vps[st * 2 + hf][:].rearrange("p (h d) -> p h d", d=D)
                nc.vector.tensor_copy(Vaug[:, st, hf * 8:(hf + 1) * 8, 0:D], src)
        nc.vector.memset(Vaug[:, :, :, D:D + 1], 1.0)

        def proj(db):
            qp = ps([128, R])
            for et in range(8):
                nc.tensor.matmul(qp[:], qk_views[db][:, 0, et, :],
                                 xT[:, et, 64:64 + R],
                                 start=(et == 0), stop=(et == 7))
            nc.scalar.add(QT[:, db, :], qp[:], bq_sb[:, db:db + 1])
            kp = ps([128, HALO])
            for et in range(8):
                nc.tensor.matmul(kp[:], qk_views[db][:, 1, et, :],
                                 xT[:, et, :], start=(et == 0), stop=False)
            nc.tensor.matmul(kp[:], bk_sb[0:1, db * 128:(db + 1) * 128],
                             valid_sb[0:1, :], start=False, stop=True)
            nc.scalar.copy(KT[:, db, :], kp[:])

        def pv_flush(pr):
            """PV matmuls + normalization scales for head pair db (one bank)."""
            db, ptl = pr
            pv = ps([128, NQB, 2 * (D + 1)])
            for qblk in range(NQB):
                for i, h in enumerate((2 * db, 2 * db + 1)):
                    off = i * (D + 1)
                    for cblk in range(2):
                        quad = qblk * 2 + cblk
                        nc.tensor.matmul(pv[:, qblk, off:off + D + 1],
                                         ptl[h][:, quad * 128:(quad + 1) * 128],
                                         Vaug[:, qblk + cblk, h, :],
                                         start=(qblk == 0 and i == 0 and cblk == 0),
                                         stop=False)
            bc_pair = bc_sb[0:1, 2 * db:2 * db + 2, :]
            for qblk in range(NQB):
                nc.tensor.matmul(pv[:, qblk, :], ones_sb[0:1, :],
                                 bc_pair.rearrange("o h d -> o (h d)"),
                                 start=False, stop=(qblk == NQB - 1))
            # one reciprocal over all four z columns, two broadcast scales
            zinv = zpool.tile([128, NQB, 2], F32, tag="z", name="zinv")
            nc.vector.reciprocal(zinv[:], pv[:, :, D::D + 1])
            for qblk in range(NQB):
                src = pv[:, qblk, :].rearrange("p (w d) -> p w d", d=D + 1)
                nc.vector.tensor_tensor(
                    Asc[:, qblk, 2 * db * D:(2 * db + 2) * D]
                    .rearrange("p (w d) -> p w d", d=D),
                    src[:, :, 0:D],
                    zinv[:, qblk, :, None].to_broadcast([128, 2, D]),
                    mybir.AluOpType.mult)

        def a_transpose(db):
            """Asc block of head pair db -> AT via PE transpose."""
            for qblk in range(NQB):
                tp = ps([128, 128], BF16)
                nc.tensor.transpose(tp[:], Asc[:, qblk, db * 128:(db + 1) * 128],
                                    identity[:])
                nc.scalar.copy(AT[:, db, qblk * 128:(qblk + 1) * 128], tp[:])

        def outproj(at_list, start, stop):
            for at in at_list:
                for qblk in range(NQB):
                    for hf in range(2):
                        nc.tensor.matmul(ops[qblk * 2 + hf][:],
                                         AT[:, at, qblk * 128:(qblk + 1) * 128],
                                         wo_t[:, at, hf * 512:(hf + 1) * 512],
                                         start=(start and at == at_list[0]),
                                         stop=False)
            if stop:
                for qblk in range(NQB):
                    for hf in range(2):
                        nc.tensor.matmul(ops[qblk * 2 + hf][:], ones_sb[0:1, :],
                                         bo_sb[0:1, hf * 512:(hf + 1) * 512],
                                         start=False, stop=True)

        def scores(db):
            # head A on PE rows 0-63, head B on rows 64-127 — interleaved
            # emission so the two row-groups run concurrently.
            sps = {h: ps([128, 512]) for h in (2 * db, 2 * db + 1)}
            for quad in range(4):
                qblk, cblk = quad // 2, quad % 2
                for i, h in enumerate((2 * db, 2 * db + 1)):
                    rr = i * 64
                    nc.tensor.matmul(
                        sps[h][:, quad * 128:(quad + 1) * 128],
                        KT[rr:rr + 64, db,
                           (qblk + cblk) * 128:(qblk + cblk + 1) * 128],
                        QT[rr:rr + 64, db, qblk * 128:(qblk + 1) * 128],
                        start=(quad == 0), stop=(quad == 3))
            ptl = {}
            for h in (2 * db, 2 * db + 1):
                et_ = epool.tile([128, 512], F32, tag="e", name="et_")
                nc.scalar.activation(et_[:], sps[h][:],
                                     mybir.ActivationFunctionType.Exp)
                pt = ppool.tile([128, 512], BF16, tag="p", name="pt")
                nc.vector.scalar_tensor_tensor(
                    pt[:], et_[:], -1.0, m4[:],
                    mybir.AluOpType.add, mybir.AluOpType.mult)
                ptl[h] = pt
            return ptl

        pend = {}   # db -> ptl, awaiting flush
        for r in range(8):
            proj(r)
            if r >= 2:
                pv_flush((r - 2, pend.pop(r - 2)))
            pend[r] = scores(r)
            if r >= 3:
                a_transpose(r - 3)

        # tail: flush 6 and 7; output projection blocks 0-5 overlap them
        ops = [ps([128, 512]) for _ in range(2 * NQB)]
        pv_flush((6, pend.pop(6)))
        a_transpose(5)
        outproj([0, 1, 2, 3, 4, 5], start=True, stop=False)
        a_transpose(6)
        pv_flush((7, pend.pop(7)))
        outproj([6], start=False, stop=False)
        a_transpose(7)
        outproj([7], start=False, stop=True)

        for qblk in range(NQB):
            ob = obpool.tile([128, E], BF16, tag="ob")
            nc.vector.tensor_copy(ob[:, 0:512], ops[qblk * 2][:])
            nc.scalar.copy(ob[:, 512:1024], ops[qblk * 2 + 1][:])
            nc.scalar.dma_start(out_d[qblk * 128:(qblk + 1) * 128, :], ob[:])

    nc.compile()
    return nc


_NC = None


def get_nc():
    global _NC
    if _NC is None:
        _NC = build_graph()
    return _NC


def make_in_maps(x, Wq, bq, Wk, bk, Wv, bv, Wo, bo):
    f = lambda a: np.ascontiguousarray(np.asarray(a, dtype=np.float32))
    bf = lambda a: np.ascontiguousarray(
        np.asarray(a, dtype=np.float32).astype(NPBF16))
    x2 = f(x).reshape(N, E)
    Wv32, Wo32 = f(Wv), f(Wo)
    ci = np.arange(128, dtype=np.float32)[:, None]  # key index c (partitions)
    qi = np.arange(128, dtype=np.float32)[None, :]  # query index q (free)
    m0 = (ci >= qi).astype(np.float32)
    m1 = (ci <= qi).astype(np.float32)
    mask4 = np.concatenate([m0, m1, m0, m1], axis=1)
    # host-folded epilogue bias: bo' = bo + bv @ Wo
    bo_row = (f(bo) + f(bv) @ Wo32).reshape(1, E)
    # host-computed global-sum row: per head [sum_n V_n | N]
    sv = (x2.sum(0, dtype=np.float32) @ Wv32).reshape(H, D)
    biascat = np.concatenate(
        [sv, np.full((H, 1), float(N), np.float32)], axis=1).reshape(1, -1)

    # db-major W: dbm[db, e_part, et*128+d] = W[et*128+e_part, db*128+d]
    def dbm(W):
        return (f(W).reshape(8, 128, 8, 128).transpose(2, 1, 0, 3)
                .reshape(8, 128, H * D))
    wqm, wkm = dbm(Wq), dbm(Wk)
    qkdb = np.concatenate([wqm, wkm], axis=2)       # [db, 128, 2048]
    r0blob = np.concatenate(
        [qkdb[0], mask4, np.eye(128, dtype=np.float32),
         f(bq).reshape(8, 128).T], axis=1)
    # [e_p, et*HALO] / [p, et*E] device layouts
    def etmaj(W):                                   # [E, X] -> [128, 8*X]
        Wf = f(W)
        return (Wf.reshape(8, 128, Wf.shape[1]).transpose(1, 0, 2)
                .reshape(128, -1))
    rowc = np.concatenate(
        [f(bk).reshape(1, E), np.zeros((1, HALO), np.float32),
         biascat, bo_row], axis=1)
    common = {
        "Wv_r": bf(etmaj(Wv)), "Wo_r": bf(etmaj(Wo)),
        "r0blob": bf(r0blob), "QKdb": bf(qkdb[1:]),
    }
    in_maps = []
    for c in range(8):
        r0 = c * R
        xh = np.zeros((HALO, E), np.float32)
        valid = np.zeros((1, HALO), np.float32)
        lo, hi = r0 - 64, r0 + R + 64
        slo, shi = max(lo, 0), min(hi, N)
        xh[slo - lo: shi - lo] = x2[slo:shi]
        valid[0, slo - lo: shi - lo] = 1.0
        rc = rowc.copy()
        rc[0, E:E + HALO] = valid
        xhT = etmaj(xh.T)                           # [128, 8*HALO]
        in_maps.append({**common, "xhT": bf(xhT), "rowc": bf(rc)})
    return in_maps


def kernel(x, Wq, bq, Wk, bk, Wv, bv, Wo, bo, _trace=False, _trace_kwargs=None):
    nc = get_nc()
    in_maps = make_in_maps(x, Wq, bq, Wk, bk, Wv, bv, Wo, bo)
    res = run_bass_kernel_spmd(nc, in_maps, list(range(8)), trace=_trace,
                               **(_trace_kwargs or {}))
    out = np.concatenate([res.results[c]["out"] for c in range(8)], axis=0)
    kernel.last_result = res
    return out[None].astype(np.float32)
